# revision 28
# baseline (speedup 1.0000x reference)
"""CosineEncoderBlock on 8 TRN2 NeuronCores — v2.

Strategy
--------
Data-parallel over the 16384 query rows (2048 per core, 4 blocks of 512);
prototypes and weights replicated.  Cosine attention is linear attention:
(q_hat @ k_hat.T) @ v == q_hat @ (k_hat.T @ v) per head; each per-head
64x64 matrix is folded with wo into one 1024x1024 W_tilde, collapsing
attention+wo into a single dense matmul on q_hat.

v2 changes vs v1:
- R=512 blocks (bigger moving operand per weight load).
- FFN1/FFN2 run in fp8 e4m3 with DoubleRow perf mode (2 contraction rows
  per PE cell): w1/w2 stored fp8 pre-scaled x64, activations quantized to
  fp8 on the fly (gelu output straight to fp8 from the scalar engine).
  w2 stays resident in SBUF (no streaming).
- The q-path layernorm needs no rstd: q_hat = z/||z_h|| is invariant to
  row scaling, so z' = (x-mu)@W' + std*cq (mean and bias folded into the
  projection as one K=2 rank-2 matmul) gives the same q_hat.  Same trick
  for the k-projection; v gets true LN via per-partition (row-major)
  rstd scaling.
- k/v are computed row-major directly (protos on partitions), removing
  all 128 PE transposes of v1.
- All activations bf16 (queries cast host-side); squares on DVE, copies
  on the scalar engine; single [16,R]/[1,R] reciprocals per block.
"""

import os

import numpy as np
from contextlib import ExitStack

DIM = 1024
HEADS = 16
DH = 64
MLP = 4096
NQ = 16384
NW = 1024
N_CORES = 8
RPC = NQ // N_CORES          # rows per core = 2048
R = 512                      # rows per block
NBLK = RPC // R              # 4 blocks per core
KC = DIM // 128              # 8 feature chunks
MC1 = MLP // 128             # 32 mlp chunks
KP1 = DIM // 256             # 4 fp8 pair chunks (FFN1 contraction)
KP2 = MLP // 256             # 16 fp8 pair chunks (FFN2 contraction)
HP = HEADS // 2              # 8 head pairs
S_W1 = 64.0
S_W2 = 64.0
LN_EPS = 1e-5

_BUILD_CACHE = {}


def _build_nc():
    import concourse.bacc as bacc
    import concourse.mybir as mybir
    import concourse.tile as tile
    from concourse.masks import make_identity

    f32 = mybir.dt.float32
    bf16 = mybir.dt.bfloat16
    f8 = mybir.dt.float8e4
    ADD = mybir.AluOpType.add
    AF = mybir.ActivationFunctionType
    DR = mybir.MatmulPerfMode.DoubleRow

    nc = bacc.Bacc("TRN2", target_bir_lowering=False, debug=False,
                   num_devices=N_CORES)

    # ---- DRAM I/O ----
    d_qT = nc.dram_tensor("qT", (128, NBLK, KC, R), bf16,
                          kind="ExternalInput").ap()
    d_pT = nc.dram_tensor("pT", (128, KC, NW), bf16, kind="ExternalInput").ap()
    d_wq = nc.dram_tensor("wq_d", (128, KC, DIM), bf16, kind="ExternalInput").ap()
    d_wk = nc.dram_tensor("wk_d", (128, KC, DIM), bf16, kind="ExternalInput").ap()
    d_wv = nc.dram_tensor("wv_d", (128, KC, DIM), bf16, kind="ExternalInput").ap()
    d_wo2 = nc.dram_tensor("wo2_d", (128, HP, DIM), bf16, kind="ExternalInput").ap()
    d_w1 = nc.dram_tensor("w1_d", (128, KP1, 2, MLP), f8, kind="ExternalInput").ap()
    d_w2 = nc.dram_tensor("w2_d", (128, KP2, 2, DIM), f8, kind="ExternalInput").ap()
    d_qc2 = nc.dram_tensor("qc2", (2, DIM), bf16, kind="ExternalInput").ap()
    d_kc2 = nc.dram_tensor("kc2", (2, DIM), bf16, kind="ExternalInput").ap()
    d_vc2 = nc.dram_tensor("vc2", (2, DIM), bf16, kind="ExternalInput").ap()
    d_b1f = nc.dram_tensor("b1f_c", (128, MC1), f32, kind="ExternalInput").ap()
    d_b2 = nc.dram_tensor("b2_c", (128, KC), f32, kind="ExternalInput").ap()
    d_bo = nc.dram_tensor("bo_c", (128, KC), f32, kind="ExternalInput").ap()
    d_oha = nc.dram_tensor("oha", (128, KC, HEADS), bf16, kind="ExternalInput").ap()
    d_ohb = nc.dram_tensor("ohb", (HEADS, DIM), bf16, kind="ExternalInput").ap()
    d_out = nc.dram_tensor("yT", (128, KC, RPC), f32, kind="ExternalOutput").ap()

    with ExitStack() as ctx:
        tc = ctx.enter_context(tile.TileContext(nc))
        ctx.enter_context(nc.allow_low_precision(
            reason="bf16/fp8 activations by design"))
        sg = ctx.enter_context(tc.tile_pool(name="singles", bufs=1))
        mpX0 = ctx.enter_context(tc.tile_pool(name="mx", bufs=2))
        x0 = mpX0.tile([128, KC, R], bf16, tag="x")
        nc.sync.dma_start(out=x0, in_=d_qT[:, 0])

        # --- early residents (wq for the block-0 head start) ---
        wqS = sg.tile([128, KC, DIM], bf16)
        nc.sync.dma_start(out=wqS, in_=d_wq)
        wtS = sg.tile([128, KC, DIM], bf16)   # W_tilde, written on device
        qc2S = sg.tile([2, DIM], bf16)
        nc.sync.dma_start(out=qc2S, in_=d_qc2)
        ohaS = sg.tile([128, KC, HEADS], bf16)
        nc.sync.dma_start(out=ohaS, in_=d_oha)
        ohbS = sg.tile([HEADS, DIM], bf16)
        nc.sync.dma_start(out=ohbS, in_=d_ohb)
        b1fS = sg.tile([128, MC1], f32)
        nc.sync.dma_start(out=b1fS, in_=d_b1f)
        b2S = sg.tile([128, KC], f32)
        nc.sync.dma_start(out=b2S, in_=d_b2)
        boS = sg.tile([128, KC], f32)
        nc.sync.dma_start(out=boS, in_=d_bo)
        onecS = sg.tile([128, 1], bf16)
        nc.vector.memset(onecS, 1.0)
        onerS = sg.tile([1, 128], bf16)
        nc.vector.memset(onerS, 1.0)
        epsS = sg.tile([1, 1], f32)
        nc.vector.memset(epsS, LN_EPS)
        identS = sg.tile([128, 128], bf16)
        make_identity(nc, identS)

        # PSUM pools: psum2 tags {mm, bc} x2 bufs; psum1 tags
        # {st, ss, r2b, m2b} x1 buf  -> 8 banks total.
        psum2 = ctx.enter_context(tc.tile_pool(name="psum2", bufs=2,
                                               space="PSUM"))
        psum1 = ctx.enter_context(tc.tile_pool(name="psum1", bufs=1,
                                               space="PSUM"))

        # long-lived activation pools (span shared phase + main loop)
        mpX = mpX0                                                 # x
        mpQ = ctx.enter_context(tc.tile_pool(name="mq", bufs=1))   # zq, qh
        mpS = ctx.enter_context(tc.tile_pool(name="ms", bufs=2))   # small rot
        mpC = ctx.enter_context(tc.tile_pool(name="mc", bufs=1))   # stat scratch

        def row_stats(xap_of, N, sq_pool, sq_tag):
            # s1 = 1.x, s2 = 1.x^2 as two sequential matmul groups in
            # separate PSUM banks (interleaved same-bank groups corrupt
            # has_written).
            s1 = psum1.tile([1, N], f32, tag="st")
            s2 = psum1.tile([1, N], f32, tag="ss")
            for c in range(KC):
                nc.tensor.matmul(s1, lhsT=onecS, rhs=xap_of(c),
                                 start=(c == 0), stop=(c == KC - 1))
            for c in range(KC):
                x2 = sq_pool.tile([128, N], bf16, tag=sq_tag)
                nc.gpsimd.tensor_mul(out=x2, in0=xap_of(c), in1=xap_of(c))
                nc.tensor.matmul(s2, lhsT=onecS, rhs=x2,
                                 start=(c == 0), stop=(c == KC - 1))
            return s1, s2

        def derive_mu_std(s1, s2, musd, tmp_pool):
            nc.vector.tensor_scalar_mul(out=musd[0:1, :], in0=s1,
                                        scalar1=1.0 / DIM)
            ex2 = tmp_pool.tile([1, R], f32, tag="ex2")
            nc.vector.tensor_scalar_mul(out=ex2, in0=s2, scalar1=1.0 / DIM)
            mq = tmp_pool.tile([1, R], f32, tag="mq")
            nc.vector.tensor_mul(out=mq, in0=musd[0:1, :], in1=musd[0:1, :])
            var = tmp_pool.tile([1, R], f32, tag="var")
            nc.vector.tensor_sub(out=var, in0=ex2, in1=mq)
            stdt = tmp_pool.tile([1, R], bf16, tag="stdt")
            nc.scalar.activation(out=stdt, in_=var, func=AF.Sqrt,
                                 bias=epsS)
            # engines cannot write partition 1; bounce through a DMA
            nc.sync.dma_start(out=musd[1:2, :], in_=stdt)

        # ---- stats + q-projection for one block (needs only wq) ----
        def emit_stats_qproj(blk, x=None):
            cols = slice(blk * R, (blk + 1) * R)
            if x is None:
                x = mpX.tile([128, KC, R], bf16, tag="x")
                nc.sync.dma_start(out=x, in_=d_qT[:, blk])
            s1, s2 = row_stats(lambda c: x[:, c, :], R, mpS, "x2")
            musd = mpS.tile([2, R], bf16, tag="musd")
            derive_mu_std(s1, s2, musd, mpC)
            zqS = mpQ.tile([128, KC, R], bf16, tag="zq")
            ss = psum1.tile([HEADS, R], f32, tag="ss")
            for m in range(KC):
                z = psum2.tile([128, R], f32, tag="mm")
                for k in range(KC):
                    nc.tensor.matmul(z, lhsT=wqS[:, k, m * 128:(m + 1) * 128],
                                     rhs=x[:, k, :],
                                     start=(k == 0), stop=False)
                nc.tensor.matmul(z, lhsT=qc2S[:, m * 128:(m + 1) * 128],
                                 rhs=musd, start=False, stop=True)
                nc.scalar.activation(out=zqS[:, m, :], in_=z, func=AF.Copy)
                z2 = mpS.tile([128, R], bf16, tag="z2")
                nc.gpsimd.tensor_mul(out=z2, in0=zqS[:, m, :], in1=zqS[:, m, :])
                nc.tensor.matmul(ss, lhsT=ohaS[:, m, :], rhs=z2,
                                 start=(m == 0), stop=(m == KC - 1))
            sn = mpS.tile([HEADS, R], bf16, tag="sn")
            nc.scalar.activation(out=sn, in_=ss, func=AF.Sqrt)
            snr = mpS.tile([HEADS, R], bf16, tag="snr")
            nc.vector.reciprocal(out=snr, in_=sn)
            qh = mpQ.tile([128, KC, R], bf16, tag="qh")
            for m in range(KC):
                cb = psum2.tile([128, R], f32, tag="bc")
                nc.tensor.matmul(cb, lhsT=ohbS[:, m * 128:(m + 1) * 128],
                                 rhs=snr, start=True, stop=True)
                nc.vector.tensor_mul(out=qh[:, m, :], in0=zqS[:, m, :], in1=cb)
            return x, qh

        blk0_x, blk0_qh = emit_stats_qproj(0, x=x0)

        # ============ shared phase: prototypes -> W_tilde ============
        spo_cm = tc.tile_pool(name="sho", bufs=1)
        spo = spo_cm.__enter__()
        kpS = spo.tile([128, KC, DIM], bf16)    # k' rows (proto-major)
        vS = spo.tile([128, KC, DIM], bf16)     # LN'd v rows
        MsbS = spo.tile([128, HP, 128], bf16)
        nc.vector.memset(MsbS, 0.0)
        musdpS = spo.tile([2, NW], bf16)        # proto mu/std rows
        prstdS = spo.tile([128, KC], f32)       # proto rstd, proto-major
        kc2S = spo.tile([2, DIM], bf16)
        nc.sync.dma_start(out=kc2S, in_=d_kc2)
        vc2S = spo.tile([2, DIM], bf16)
        nc.sync.dma_start(out=vc2S, in_=d_vc2)

        with tc.tile_pool(name="shi", bufs=1) as spi, \
             tc.tile_pool(name="sh2", bufs=2) as sp2:
            pTS = spi.tile([128, KC, NW], bf16)
            nc.sync.dma_start(out=pTS, in_=d_pT)
            wkS = spi.tile([128, KC, DIM], bf16, tag="wkv")
            nc.sync.dma_start(out=wkS, in_=d_wk)
            wvS = spi.tile([128, KC, DIM], bf16, tag="wv2")
            nc.sync.dma_start(out=wvS, in_=d_wv)

            # proto stats, feature-major, two 512-col halves
            for nh in range(2):
                cols = slice(nh * 512, (nh + 1) * 512)
                s1 = psum1.tile([1, 512], f32, tag="st")
                s2 = psum1.tile([1, 512], f32, tag="ss")
                for dc in range(KC):
                    nc.tensor.matmul(s1, lhsT=onecS, rhs=pTS[:, dc, cols],
                                     start=(dc == 0), stop=(dc == KC - 1))
                for dc in range(KC):
                    x2 = sp2.tile([128, 512], bf16, tag="px2")
                    nc.vector.tensor_mul(out=x2, in0=pTS[:, dc, cols],
                                         in1=pTS[:, dc, cols])
                    nc.tensor.matmul(s2, lhsT=onecS, rhs=x2,
                                     start=(dc == 0), stop=(dc == KC - 1))
                nc.vector.tensor_scalar_mul(out=musdpS[0:1, cols], in0=s1,
                                            scalar1=1.0 / DIM)
                ex2 = spi.tile([1, 512], f32, tag="pex2")
                nc.vector.tensor_scalar_mul(out=ex2, in0=s2, scalar1=1.0 / DIM)
                mq = spi.tile([1, 512], f32, tag="pmq")
                nc.vector.tensor_mul(out=mq, in0=musdpS[0:1, cols],
                                     in1=musdpS[0:1, cols])
                var = spi.tile([1, 512], f32, tag="pvar")
                nc.vector.tensor_sub(out=var, in0=ex2, in1=mq)
                pstd = spi.tile([1, 512], bf16, tag="pstd")
                nc.scalar.activation(out=pstd, in_=var,
                                     func=AF.Sqrt, bias=epsS)
                nc.sync.dma_start(out=musdpS[1:2, cols], in_=pstd)
            # transpose std to proto-major, reciprocal -> prstdS
            for c in range(KC):
                tp = psum1.tile([128, 2], bf16, tag="st")
                nc.tensor.transpose(tp,
                                    in_=musdpS[:, c * 128:(c + 1) * 128],
                                    identity=identS[0:2, 0:2])
                nc.vector.reciprocal(out=prstdS[:, c:c + 1], in_=tp[:, 1:2])

            # k/v projections, row-major output; 4 rhs per lhsT
            for pc in range(KC):
                for half in range(2):
                    fcols = slice(half * 512, (half + 1) * 512)
                    zk = psum2.tile([128, 512], f32, tag="mm")
                    zv = psum2.tile([128, 512], f32, tag="bc")
                    for dc in range(KC):
                        lhs = pTS[:, dc, pc * 128:(pc + 1) * 128]
                        nc.tensor.matmul(zk, lhsT=lhs,
                                         rhs=wkS[:, dc, fcols],
                                         start=(dc == 0), stop=False)
                        nc.tensor.matmul(zv, lhsT=lhs,
                                         rhs=wvS[:, dc, fcols],
                                         start=(dc == 0), stop=False)
                    lhs2 = musdpS[:, pc * 128:(pc + 1) * 128]
                    nc.tensor.matmul(zk, lhsT=lhs2, rhs=kc2S[:, fcols],
                                     start=False, stop=True)
                    nc.tensor.matmul(zv, lhsT=lhs2, rhs=vc2S[:, fcols],
                                     start=False, stop=True)
                    nc.scalar.activation(out=kpS[:, pc, fcols], in_=zk,
                                         func=AF.Copy)
                    nc.vector.tensor_scalar_mul(out=vS[:, pc, fcols], in0=zv,
                                                scalar1=prstdS[:, pc:pc + 1])

            # k_hat: per-head L2 normalize (in-place scale of kpS)
            for pc in range(KC):
                k2 = sp2.tile([128, HEADS, DH], bf16, tag="pk2")
                kview = kpS[:, pc, :].rearrange("p (h d) -> p h d", h=HEADS)
                nc.vector.tensor_mul(out=k2, in0=kview, in1=kview)
                kss = sp2.tile([128, HEADS], f32, tag="pkss")
                nc.vector.reduce_sum(out=kss, in_=k2,
                                     axis=mybir.AxisListType.X)
                ksq = sp2.tile([128, HEADS], f32, tag="pksq")
                nc.scalar.activation(out=ksq, in_=kss, func=AF.Sqrt)
                ksn = sp2.tile([128, HEADS], f32, tag="pksn")
                nc.vector.reciprocal(out=ksn, in_=ksq)
                for h in range(HEADS):
                    hsl = slice(h * DH, (h + 1) * DH)
                    if h % 2 == 0:
                        nc.vector.tensor_scalar_mul(
                            out=kpS[:, pc, hsl], in0=kpS[:, pc, hsl],
                            scalar1=ksn[:, h:h + 1])
                    else:
                        nc.scalar.activation(
                            out=kpS[:, pc, hsl], in_=kpS[:, pc, hsl],
                            func=AF.Copy, scale=ksn[:, h:h + 1])

            # M' per head pair: zm = v^T k_hat (diag 64x64 blocks kept)
            for hp in range(HP):
                zm = psum2.tile([128, 128], f32, tag="mm")
                for pc in range(KC):
                    nc.tensor.matmul(zm,
                                     lhsT=vS[:, pc, hp * 128:(hp + 1) * 128],
                                     rhs=kpS[:, pc, hp * 128:(hp + 1) * 128],
                                     start=(pc == 0), stop=(pc == KC - 1))
                nc.scalar.activation(out=MsbS[0:64, hp, 0:64],
                                     in_=zm[0:64, 0:64], func=AF.Copy)
                nc.scalar.activation(out=MsbS[64:128, hp, 64:128],
                                     in_=zm[64:128, 64:128], func=AF.Copy)

        # W_tilde (reuses freed staging space)
        with tc.tile_pool(name="sho2", bufs=1) as spw:
            wo2S = spw.tile([128, HP, DIM], bf16)
            nc.sync.dma_start(out=wo2S, in_=d_wo2)
            for hp in range(HP):
                for half in range(2):
                    fcols = slice(half * 512, (half + 1) * 512)
                    zw = psum2.tile([128, 512], f32, tag="bc")
                    nc.tensor.matmul(zw, lhsT=MsbS[:, hp, :],
                                     rhs=wo2S[:, hp, fcols],
                                     start=True, stop=True)
                    nc.scalar.activation(out=wtS[:, hp, fcols], in_=zw,
                                         func=AF.Copy)
        spo_cm.__exit__(None, None, None)

        # late residents (fp8 ffn weights) + main-loop activations
        lr = ctx.enter_context(tc.tile_pool(name="lateres", bufs=1))
        w1S = lr.tile([128, KP1, 2, MLP], f8)
        nc.sync.dma_start(out=w1S, in_=d_w1)
        w2S = lr.tile([128, KP2, 2, DIM], f8)
        nc.sync.dma_start(out=w2S, in_=d_w2)

        # ============ main loop over query blocks ============
        # FFN of block b-1 runs inside iteration b: it gives the tensor
        # engine ~55us of independent DR matmuls to execute while DVE
        # works through block b's LN2 chain (evac/square/derive/recip).
        mpM = ctx.enter_context(tc.tile_pool(name="mmain", bufs=1))

        def emit_ffn1(xh2, g):
            for m in range(MC1):
                z = psum2.tile([128, R], f32, tag="mm")
                for kp in range(KP1):
                    nc.tensor.matmul(
                        z, lhsT=w1S[:, kp, :, m * 128:(m + 1) * 128],
                        rhs=xh2[:, 2 * kp:2 * kp + 2, :],
                        start=(kp == 0), stop=(kp == KP1 - 1), perf_mode=DR)
                nc.scalar.activation(out=g[:, m, :], in_=z, func=AF.Gelu,
                                     bias=b1fS[:, m:m + 1], scale=1.0 / S_W1)

        def emit_ffn2(g, a, cols):
            for n in range(KC):
                z = psum2.tile([128, R], f32, tag="mm")
                for kp in range(KP2):
                    nc.tensor.matmul(
                        z, lhsT=w2S[:, kp, :, n * 128:(n + 1) * 128],
                        rhs=g[:, 2 * kp:2 * kp + 2, :],
                        start=(kp == 0), stop=(kp == KP2 - 1), perf_mode=DR)
                y1 = mpS.tile([128, R], bf16, tag="y1")
                nc.scalar.activation(out=y1, in_=z, func=AF.Identity,
                                     bias=b2S[:, n:n + 1], scale=1.0 / S_W2)
                yo = mpS.tile([128, R], f32, tag="yo")
                nc.gpsimd.tensor_add(out=yo, in0=y1, in1=a[:, n, :])
                nc.sync.dma_start(out=d_out[:, n, cols], in_=yo)

        x, qh = blk0_x, blk0_qh
        prev = None            # (xh2, a, cols) of the previous block
        for blk in range(NBLK):
            cols = slice(blk * R, (blk + 1) * R)

            # attention + wo fold: a = qh @ W_tilde + bo + x
            a = mpM.tile([128, KC, R], bf16, tag=f"a{blk % 2}")
            for m in range(KC):
                z = psum2.tile([128, R], f32, tag="mm")
                for k in range(KC):
                    nc.tensor.matmul(z, lhsT=wtS[:, k, m * 128:(m + 1) * 128],
                                     rhs=qh[:, k, :],
                                     start=(k == 0), stop=(k == KC - 1))
                nc.vector.scalar_tensor_tensor(
                    out=a[:, m, :], in0=z, scalar=boS[:, m:m + 1],
                    in1=x[:, m, :], op0=ADD, op1=ADD)

            g = mpM.tile([128, MC1, R], f8, tag="g")
            if prev is not None:
                emit_ffn1(prev[0], g)

            s1b, s2b = row_stats(lambda c: a[:, c, :], R, mpS, "x2")
            mu2 = mpC.tile([1, R], bf16, tag="mu2")
            nc.vector.tensor_scalar_mul(out=mu2, in0=s1b, scalar1=1.0 / DIM)
            ex2b = mpC.tile([1, R], f32, tag="ex2")
            nc.vector.tensor_scalar_mul(out=ex2b, in0=s2b, scalar1=1.0 / DIM)
            mqb = mpC.tile([1, R], f32, tag="mq")
            nc.vector.tensor_mul(out=mqb, in0=mu2, in1=mu2)
            varb = mpC.tile([1, R], f32, tag="var")
            nc.vector.tensor_sub(out=varb, in0=ex2b, in1=mqb)
            sq2 = mpC.tile([1, R], f32, tag="bsq")
            nc.scalar.activation(out=sq2, in_=varb, func=AF.Sqrt, bias=epsS)
            rstd2 = mpC.tile([1, R], bf16, tag="rs2")
            nc.vector.reciprocal(out=rstd2, in_=sq2)
            mr2 = mpC.tile([1, R], bf16, tag="mr2")
            nc.vector.tensor_mul(out=mr2, in0=mu2, in1=rstd2)

            if prev is not None:
                emit_ffn2(g, prev[1], prev[2])

            r2b = psum1.tile([128, R], f32, tag="r2b")
            nc.tensor.matmul(r2b, lhsT=onerS, rhs=rstd2,
                             start=True, stop=True)
            m2b = psum1.tile([128, R], f32, tag="m2b")
            nc.tensor.matmul(m2b, lhsT=onerS, rhs=mr2,
                             start=True, stop=True)
            # bounce broadcasts to SBUF (negate m2b) so the LN2 apply can
            # run on the otherwise-idle gpsimd engine (no PSUM port there)
            r2bS = mpC.tile([128, R], bf16, tag="r2bS")
            nc.scalar.activation(out=r2bS, in_=r2b, func=AF.Copy)
            m2nS = mpC.tile([128, R], bf16, tag="m2nS")
            nc.scalar.activation(out=m2nS, in_=m2b, func=AF.Copy, scale=-1.0)

            if blk + 1 < NBLK:
                nx, nqh = emit_stats_qproj(blk + 1)

            # xh2 for this block (consumed by FFN1 next iteration)
            xh2 = mpM.tile([128, KC, R], f8, tag="xh2")
            for c in range(KC):
                t = mpC.tile([128, R], bf16, tag="lnt")
                nc.gpsimd.tensor_mul(out=t, in0=a[:, c, :], in1=r2bS)
                nc.gpsimd.tensor_add(out=xh2[:, c, :], in0=t, in1=m2nS)

            prev = (xh2, a, cols)
            if blk + 1 < NBLK:
                x, qh = nx, nqh

        # epilogue: FFN of the last block
        g = mpM.tile([128, MC1, R], f8, tag="g")
        emit_ffn1(prev[0], g)
        emit_ffn2(g, prev[1], prev[2])

    nc.compile()
    return nc


def _prep_inputs(inputs):
    import ml_dtypes

    bf16 = ml_dtypes.bfloat16
    f8 = ml_dtypes.float8_e4m3
    f32 = np.float32

    queries = np.asarray(inputs["queries"], dtype=f32)
    prototypes = np.asarray(inputs["prototypes"], dtype=f32)
    ln1_w = np.asarray(inputs["ln1_w"], dtype=f32)
    ln1_b = np.asarray(inputs["ln1_b"], dtype=f32)
    wq = np.asarray(inputs["wq"], dtype=f32)
    wk = np.asarray(inputs["wk"], dtype=f32)
    wv = np.asarray(inputs["wv"], dtype=f32)
    wo = np.asarray(inputs["wo"], dtype=f32)
    bo = np.asarray(inputs["bo"], dtype=f32)
    ln2_w = np.asarray(inputs["ln2_w"], dtype=f32)
    ln2_b = np.asarray(inputs["ln2_b"], dtype=f32)
    w1 = np.asarray(inputs["w1"], dtype=f32)
    b1 = np.asarray(inputs["b1"], dtype=f32)
    w2 = np.asarray(inputs["w2"], dtype=f32)
    b2 = np.asarray(inputs["b2"], dtype=f32)

    def fm(w):  # [DIM, M] feature-major chunks -> [128, KC, M]
        return np.ascontiguousarray(
            w.reshape(KC, 128, w.shape[1]).transpose(1, 0, 2))

    def cols128(v, nchunks):
        return np.ascontiguousarray(v.reshape(nchunks, 128).T).astype(f32)

    wq_d = fm((wq * ln1_w[:, None]).astype(bf16))
    wk_d = fm((wk * ln1_w[:, None]).astype(bf16))
    wv_d = fm((wv * ln1_w[:, None]).astype(bf16))
    w1_s = np.clip(w1 * ln2_w[:, None] * S_W1, -240, 240).astype(f8)
    w2_s = np.clip(w2 * S_W2, -240, 240).astype(f8)
    w1_d = np.ascontiguousarray(
        w1_s.reshape(KP1, 2, 128, MLP).transpose(2, 0, 1, 3))
    w2_d = np.ascontiguousarray(
        w2_s.reshape(KP2, 2, 128, DIM).transpose(2, 0, 1, 3))
    wo2_d = np.ascontiguousarray(
        wo.astype(bf16).reshape(HP, 128, DIM).transpose(1, 0, 2))

    qc2 = np.stack([-(ln1_w @ wq), ln1_b @ wq]).astype(bf16)
    kc2 = np.stack([-(ln1_w @ wk), ln1_b @ wk]).astype(bf16)
    vc2 = np.stack([-(ln1_w @ wv), ln1_b @ wv]).astype(bf16)
    b1f = (b1 + ln2_b @ w1).astype(f32)

    oha = np.zeros((DIM, HEADS), dtype=f32)
    for h in range(HEADS):
        oha[h * DH:(h + 1) * DH, h] = 1.0
    oha_d = fm(oha.astype(bf16))
    ohb_d = np.ascontiguousarray(oha.T).astype(bf16)

    pT = fm(prototypes.T.astype(bf16))

    common = {
        "pT": pT, "wq_d": wq_d, "wk_d": wk_d, "wv_d": wv_d,
        "wo2_d": wo2_d, "w1_d": w1_d, "w2_d": w2_d,
        "qc2": qc2, "kc2": kc2, "vc2": vc2,
        "b1f_c": cols128(b1f, MC1), "b2_c": cols128(b2, KC),
        "bo_c": cols128(bo, KC), "oha": oha_d, "ohb": ohb_d,
    }
    qT = queries.T.astype(bf16)                    # [DIM, NQ]
    in_maps = []
    for c in range(N_CORES):
        m = dict(common)
        qc = fm(np.ascontiguousarray(qT[:, c * RPC:(c + 1) * RPC]))
        # [128, KC, RPC] -> per-block contiguous [128, NBLK, KC, R]
        m["qT"] = np.ascontiguousarray(
            qc.reshape(128, KC, NBLK, R).transpose(0, 2, 1, 3))
        in_maps.append(m)
    return in_maps


def kernel(**inputs):
    from concourse.bass_utils import run_bass_kernel_spmd

    in_maps = _prep_inputs(inputs)

    if "nc" not in _BUILD_CACHE:
        _BUILD_CACHE["nc"] = _build_nc()
    nc = _BUILD_CACHE["nc"]

    trace = bool(os.environ.get("KERNEL_TRACE"))
    res = run_bass_kernel_spmd(nc, in_maps, core_ids=list(range(N_CORES)),
                               trace=trace)
    _BUILD_CACHE["last_res"] = res
    # yT [128, KC, RPC] -> rows
    parts = []
    for c in range(N_CORES):
        yT = res.results[c]["yT"]
        parts.append(yT.transpose(2, 1, 0).reshape(RPC, DIM))
    return np.ascontiguousarray(np.concatenate(parts, axis=0))


# revision 29
# speedup vs baseline: 1.0408x; 1.0408x over previous
"""CosineEncoderBlock on 8 TRN2 NeuronCores — v2.

Strategy
--------
Data-parallel over the 16384 query rows (2048 per core, 4 blocks of 512);
prototypes and weights replicated.  Cosine attention is linear attention:
(q_hat @ k_hat.T) @ v == q_hat @ (k_hat.T @ v) per head; each per-head
64x64 matrix is folded with wo into one 1024x1024 W_tilde, collapsing
attention+wo into a single dense matmul on q_hat.

v2 changes vs v1:
- R=512 blocks (bigger moving operand per weight load).
- FFN1/FFN2 run in fp8 e4m3 with DoubleRow perf mode (2 contraction rows
  per PE cell): w1/w2 stored fp8 pre-scaled x64, activations quantized to
  fp8 on the fly (gelu output straight to fp8 from the scalar engine).
  w2 stays resident in SBUF (no streaming).
- The q-path layernorm needs no rstd: q_hat = z/||z_h|| is invariant to
  row scaling, so z' = (x-mu)@W' + std*cq (mean and bias folded into the
  projection as one K=2 rank-2 matmul) gives the same q_hat.  Same trick
  for the k-projection; v gets true LN via per-partition (row-major)
  rstd scaling.
- k/v are computed row-major directly (protos on partitions), removing
  all 128 PE transposes of v1.
- All activations bf16 (queries cast host-side); squares on DVE, copies
  on the scalar engine; single [16,R]/[1,R] reciprocals per block.
"""

import os

import numpy as np
from contextlib import ExitStack

DIM = 1024
HEADS = 16
DH = 64
MLP = 4096
NQ = 16384
NW = 1024
N_CORES = 8
RPC = NQ // N_CORES          # rows per core = 2048
R = 512                      # rows per block
NBLK = RPC // R              # 4 blocks per core
KC = DIM // 128              # 8 feature chunks
MC1 = MLP // 128             # 32 mlp chunks
KP1 = DIM // 256             # 4 fp8 pair chunks (FFN1 contraction)
KP2 = MLP // 256             # 16 fp8 pair chunks (FFN2 contraction)
HP = HEADS // 2              # 8 head pairs
S_W1 = 64.0
S_W2 = 64.0
LN_EPS = 1e-5

_BUILD_CACHE = {}


def _build_nc():
    import concourse.bacc as bacc
    import concourse.mybir as mybir
    import concourse.tile as tile
    from concourse.masks import make_identity

    f32 = mybir.dt.float32
    bf16 = mybir.dt.bfloat16
    f8 = mybir.dt.float8e4
    ADD = mybir.AluOpType.add
    AF = mybir.ActivationFunctionType
    DR = mybir.MatmulPerfMode.DoubleRow

    nc = bacc.Bacc("TRN2", target_bir_lowering=False, debug=False,
                   num_devices=N_CORES)

    # ---- DRAM I/O ----
    d_qT = nc.dram_tensor("qT", (128, NBLK, KC, R), bf16,
                          kind="ExternalInput").ap()
    d_pT = nc.dram_tensor("pT", (128, KC, NW), bf16, kind="ExternalInput").ap()
    d_wq = nc.dram_tensor("wq_d", (128, KC, DIM), bf16, kind="ExternalInput").ap()
    d_wk = nc.dram_tensor("wk_d", (128, KC, DIM), bf16, kind="ExternalInput").ap()
    d_wv = nc.dram_tensor("wv_d", (128, KC, DIM), bf16, kind="ExternalInput").ap()
    d_wo2 = nc.dram_tensor("wo2_d", (128, HP, DIM), bf16, kind="ExternalInput").ap()
    d_w1 = nc.dram_tensor("w1_d", (128, KP1, 2, MLP), f8, kind="ExternalInput").ap()
    d_w2 = nc.dram_tensor("w2_d", (128, KP2, 2, DIM), f8, kind="ExternalInput").ap()
    d_qc2 = nc.dram_tensor("qc2", (2, DIM), bf16, kind="ExternalInput").ap()
    d_kc2 = nc.dram_tensor("kc2", (2, DIM), bf16, kind="ExternalInput").ap()
    d_vc2 = nc.dram_tensor("vc2", (2, DIM), bf16, kind="ExternalInput").ap()
    d_b1f = nc.dram_tensor("b1f_c", (128, MC1), f32, kind="ExternalInput").ap()
    d_b2 = nc.dram_tensor("b2_c", (128, KC), f32, kind="ExternalInput").ap()
    d_bo = nc.dram_tensor("bo_c", (128, KC), f32, kind="ExternalInput").ap()
    d_oha = nc.dram_tensor("oha", (128, KC, HEADS), bf16, kind="ExternalInput").ap()
    d_ohb = nc.dram_tensor("ohb", (HEADS, DIM), bf16, kind="ExternalInput").ap()
    d_out = nc.dram_tensor("yT", (128, KC, RPC), f32, kind="ExternalOutput").ap()

    with ExitStack() as ctx:
        tc = ctx.enter_context(tile.TileContext(nc))
        ctx.enter_context(nc.allow_low_precision(
            reason="bf16/fp8 activations by design"))
        sg = ctx.enter_context(tc.tile_pool(name="singles", bufs=1))
        mpX0 = ctx.enter_context(tc.tile_pool(name="mx", bufs=2))
        x0 = mpX0.tile([128, KC, R], bf16, tag="x")
        nc.sync.dma_start(out=x0, in_=d_qT[:, 0])

        # --- early residents (wq for the block-0 head start) ---
        wqS = sg.tile([128, KC, DIM], bf16)
        nc.sync.dma_start(out=wqS, in_=d_wq)
        wtS = sg.tile([128, KC, DIM], bf16)   # W_tilde, written on device
        qc2S = sg.tile([2, DIM], bf16)
        nc.sync.dma_start(out=qc2S, in_=d_qc2)
        ohaS = sg.tile([128, KC, HEADS], bf16)
        nc.sync.dma_start(out=ohaS, in_=d_oha)
        ohbS = sg.tile([HEADS, DIM], bf16)
        nc.sync.dma_start(out=ohbS, in_=d_ohb)
        b1fS = sg.tile([128, MC1], f32)
        nc.sync.dma_start(out=b1fS, in_=d_b1f)
        b2S = sg.tile([128, KC], f32)
        nc.sync.dma_start(out=b2S, in_=d_b2)
        boS = sg.tile([128, KC], f32)
        nc.sync.dma_start(out=boS, in_=d_bo)
        onecS = sg.tile([128, 1], bf16)
        nc.vector.memset(onecS, 1.0)
        onerS = sg.tile([1, 128], bf16)
        nc.vector.memset(onerS, 1.0)
        epsS = sg.tile([1, 1], f32)
        nc.vector.memset(epsS, LN_EPS)
        identS = sg.tile([128, 128], bf16)
        make_identity(nc, identS)

        # PSUM pools: psum2 tags {mm, bc} x2 bufs; psum1 tags
        # {st, ss, r2b, m2b} x1 buf  -> 8 banks total.
        psum2 = ctx.enter_context(tc.tile_pool(name="psum2", bufs=2,
                                               space="PSUM"))
        psum1 = ctx.enter_context(tc.tile_pool(name="psum1", bufs=1,
                                               space="PSUM"))

        # long-lived activation pools (span shared phase + main loop)
        mpX = mpX0                                                 # x
        mpQ = ctx.enter_context(tc.tile_pool(name="mq", bufs=1))   # zq, qh
        mpS = ctx.enter_context(tc.tile_pool(name="ms", bufs=2))   # small rot
        mpC = ctx.enter_context(tc.tile_pool(name="mc", bufs=1))   # stat scratch

        def row_stats(xap_of, N, sq_pool, sq_tag):
            # s1 = 1.x, s2 = 1.x^2 as two sequential matmul groups in
            # separate PSUM banks (interleaved same-bank groups corrupt
            # has_written).
            s1 = psum1.tile([1, N], f32, tag="st")
            s2 = psum1.tile([1, N], f32, tag="ss")
            for c in range(KC):
                nc.tensor.matmul(s1, lhsT=onecS, rhs=xap_of(c),
                                 start=(c == 0), stop=(c == KC - 1))
            for c in range(KC):
                x2 = sq_pool.tile([128, N], bf16, tag=sq_tag)
                nc.vector.tensor_mul(out=x2, in0=xap_of(c), in1=xap_of(c))
                nc.tensor.matmul(s2, lhsT=onecS, rhs=x2,
                                 start=(c == 0), stop=(c == KC - 1))
            return s1, s2

        def derive_mu_std(s1, s2, musd, tmp_pool):
            nc.vector.tensor_scalar_mul(out=musd[0:1, :], in0=s1,
                                        scalar1=1.0 / DIM)
            ex2 = tmp_pool.tile([1, R], f32, tag="ex2")
            nc.vector.tensor_scalar_mul(out=ex2, in0=s2, scalar1=1.0 / DIM)
            mq = tmp_pool.tile([1, R], f32, tag="mq")
            nc.vector.tensor_mul(out=mq, in0=musd[0:1, :], in1=musd[0:1, :])
            var = tmp_pool.tile([1, R], f32, tag="var")
            nc.vector.tensor_sub(out=var, in0=ex2, in1=mq)
            stdt = tmp_pool.tile([1, R], bf16, tag="stdt")
            nc.scalar.activation(out=stdt, in_=var, func=AF.Sqrt,
                                 bias=epsS)
            # engines cannot write partition 1; bounce through a DMA
            nc.sync.dma_start(out=musd[1:2, :], in_=stdt)

        # ---- stats + q-projection for one block (needs only wq) ----
        def emit_stats_qproj(blk, x=None):
            cols = slice(blk * R, (blk + 1) * R)
            if x is None:
                x = mpX.tile([128, KC, R], bf16, tag="x")
                nc.sync.dma_start(out=x, in_=d_qT[:, blk])
            s1, s2 = row_stats(lambda c: x[:, c, :], R, mpS, "x2")
            musd = mpS.tile([2, R], bf16, tag="musd")
            derive_mu_std(s1, s2, musd, mpC)
            zqS = mpQ.tile([128, KC, R], bf16, tag="zq")
            ss = psum1.tile([HEADS, R], f32, tag="ss")
            for m in range(KC):
                z = psum2.tile([128, R], f32, tag="mm")
                for k in range(KC):
                    nc.tensor.matmul(z, lhsT=wqS[:, k, m * 128:(m + 1) * 128],
                                     rhs=x[:, k, :],
                                     start=(k == 0), stop=False)
                nc.tensor.matmul(z, lhsT=qc2S[:, m * 128:(m + 1) * 128],
                                 rhs=musd, start=False, stop=True)
                nc.scalar.activation(out=zqS[:, m, :], in_=z, func=AF.Copy)
                z2 = mpS.tile([128, R], bf16, tag="z2")
                nc.vector.tensor_mul(out=z2, in0=zqS[:, m, :], in1=zqS[:, m, :])
                nc.tensor.matmul(ss, lhsT=ohaS[:, m, :], rhs=z2,
                                 start=(m == 0), stop=(m == KC - 1))
            sn = mpS.tile([HEADS, R], bf16, tag="sn")
            nc.scalar.activation(out=sn, in_=ss, func=AF.Sqrt)
            snr = mpS.tile([HEADS, R], bf16, tag="snr")
            nc.vector.reciprocal(out=snr, in_=sn)
            qh = mpQ.tile([128, KC, R], bf16, tag="qh")
            for m in range(KC):
                cb = psum2.tile([128, R], f32, tag="bc")
                nc.tensor.matmul(cb, lhsT=ohbS[:, m * 128:(m + 1) * 128],
                                 rhs=snr, start=True, stop=True)
                nc.vector.tensor_mul(out=qh[:, m, :], in0=zqS[:, m, :], in1=cb)
            return x, qh

        blk0_x, blk0_qh = emit_stats_qproj(0, x=x0)

        # ============ shared phase: prototypes -> W_tilde ============
        spo_cm = tc.tile_pool(name="sho", bufs=1)
        spo = spo_cm.__enter__()
        kpS = spo.tile([128, KC, DIM], bf16)    # k' rows (proto-major)
        vS = spo.tile([128, KC, DIM], bf16)     # LN'd v rows
        MsbS = spo.tile([128, HP, 128], bf16)
        nc.vector.memset(MsbS, 0.0)
        musdpS = spo.tile([2, NW], bf16)        # proto mu/std rows
        prstdS = spo.tile([128, KC], f32)       # proto rstd, proto-major
        kc2S = spo.tile([2, DIM], bf16)
        nc.sync.dma_start(out=kc2S, in_=d_kc2)
        vc2S = spo.tile([2, DIM], bf16)
        nc.sync.dma_start(out=vc2S, in_=d_vc2)

        with tc.tile_pool(name="shi", bufs=1) as spi, \
             tc.tile_pool(name="sh2", bufs=2) as sp2:
            pTS = spi.tile([128, KC, NW], bf16)
            nc.sync.dma_start(out=pTS, in_=d_pT)
            wkS = spi.tile([128, KC, DIM], bf16, tag="wkv")
            nc.sync.dma_start(out=wkS, in_=d_wk)
            wvS = spi.tile([128, KC, DIM], bf16, tag="wv2")
            nc.sync.dma_start(out=wvS, in_=d_wv)

            # proto stats, feature-major, two 512-col halves
            for nh in range(2):
                cols = slice(nh * 512, (nh + 1) * 512)
                s1 = psum1.tile([1, 512], f32, tag="st")
                s2 = psum1.tile([1, 512], f32, tag="ss")
                for dc in range(KC):
                    nc.tensor.matmul(s1, lhsT=onecS, rhs=pTS[:, dc, cols],
                                     start=(dc == 0), stop=(dc == KC - 1))
                for dc in range(KC):
                    x2 = sp2.tile([128, 512], bf16, tag="px2")
                    nc.vector.tensor_mul(out=x2, in0=pTS[:, dc, cols],
                                         in1=pTS[:, dc, cols])
                    nc.tensor.matmul(s2, lhsT=onecS, rhs=x2,
                                     start=(dc == 0), stop=(dc == KC - 1))
                nc.vector.tensor_scalar_mul(out=musdpS[0:1, cols], in0=s1,
                                            scalar1=1.0 / DIM)
                ex2 = spi.tile([1, 512], f32, tag="pex2")
                nc.vector.tensor_scalar_mul(out=ex2, in0=s2, scalar1=1.0 / DIM)
                mq = spi.tile([1, 512], f32, tag="pmq")
                nc.vector.tensor_mul(out=mq, in0=musdpS[0:1, cols],
                                     in1=musdpS[0:1, cols])
                var = spi.tile([1, 512], f32, tag="pvar")
                nc.vector.tensor_sub(out=var, in0=ex2, in1=mq)
                pstd = spi.tile([1, 512], bf16, tag="pstd")
                nc.scalar.activation(out=pstd, in_=var,
                                     func=AF.Sqrt, bias=epsS)
                nc.sync.dma_start(out=musdpS[1:2, cols], in_=pstd)
            # transpose std to proto-major, reciprocal -> prstdS
            for c in range(KC):
                tp = psum1.tile([128, 2], bf16, tag="st")
                nc.tensor.transpose(tp,
                                    in_=musdpS[:, c * 128:(c + 1) * 128],
                                    identity=identS[0:2, 0:2])
                nc.vector.reciprocal(out=prstdS[:, c:c + 1], in_=tp[:, 1:2])

            # k/v projections, row-major output; 4 rhs per lhsT
            for pc in range(KC):
                for half in range(2):
                    fcols = slice(half * 512, (half + 1) * 512)
                    zk = psum2.tile([128, 512], f32, tag="mm")
                    zv = psum2.tile([128, 512], f32, tag="bc")
                    for dc in range(KC):
                        lhs = pTS[:, dc, pc * 128:(pc + 1) * 128]
                        nc.tensor.matmul(zk, lhsT=lhs,
                                         rhs=wkS[:, dc, fcols],
                                         start=(dc == 0), stop=False)
                        nc.tensor.matmul(zv, lhsT=lhs,
                                         rhs=wvS[:, dc, fcols],
                                         start=(dc == 0), stop=False)
                    lhs2 = musdpS[:, pc * 128:(pc + 1) * 128]
                    nc.tensor.matmul(zk, lhsT=lhs2, rhs=kc2S[:, fcols],
                                     start=False, stop=True)
                    nc.tensor.matmul(zv, lhsT=lhs2, rhs=vc2S[:, fcols],
                                     start=False, stop=True)
                    nc.scalar.activation(out=kpS[:, pc, fcols], in_=zk,
                                         func=AF.Copy)
                    nc.vector.tensor_scalar_mul(out=vS[:, pc, fcols], in0=zv,
                                                scalar1=prstdS[:, pc:pc + 1])

            # k_hat: per-head L2 normalize (in-place scale of kpS)
            for pc in range(KC):
                k2 = sp2.tile([128, HEADS, DH], bf16, tag="pk2")
                kview = kpS[:, pc, :].rearrange("p (h d) -> p h d", h=HEADS)
                nc.vector.tensor_mul(out=k2, in0=kview, in1=kview)
                kss = sp2.tile([128, HEADS], f32, tag="pkss")
                nc.vector.reduce_sum(out=kss, in_=k2,
                                     axis=mybir.AxisListType.X)
                ksq = sp2.tile([128, HEADS], f32, tag="pksq")
                nc.scalar.activation(out=ksq, in_=kss, func=AF.Sqrt)
                ksn = sp2.tile([128, HEADS], f32, tag="pksn")
                nc.vector.reciprocal(out=ksn, in_=ksq)
                for h in range(HEADS):
                    hsl = slice(h * DH, (h + 1) * DH)
                    if h % 2 == 0:
                        nc.vector.tensor_scalar_mul(
                            out=kpS[:, pc, hsl], in0=kpS[:, pc, hsl],
                            scalar1=ksn[:, h:h + 1])
                    else:
                        nc.scalar.activation(
                            out=kpS[:, pc, hsl], in_=kpS[:, pc, hsl],
                            func=AF.Copy, scale=ksn[:, h:h + 1])

            # M' per head pair: zm = v^T k_hat (diag 64x64 blocks kept)
            for hp in range(HP):
                zm = psum2.tile([128, 128], f32, tag="mm")
                for pc in range(KC):
                    nc.tensor.matmul(zm,
                                     lhsT=vS[:, pc, hp * 128:(hp + 1) * 128],
                                     rhs=kpS[:, pc, hp * 128:(hp + 1) * 128],
                                     start=(pc == 0), stop=(pc == KC - 1))
                nc.scalar.activation(out=MsbS[0:64, hp, 0:64],
                                     in_=zm[0:64, 0:64], func=AF.Copy)
                nc.scalar.activation(out=MsbS[64:128, hp, 64:128],
                                     in_=zm[64:128, 64:128], func=AF.Copy)

        # W_tilde (reuses freed staging space)
        with tc.tile_pool(name="sho2", bufs=1) as spw:
            wo2S = spw.tile([128, HP, DIM], bf16)
            nc.sync.dma_start(out=wo2S, in_=d_wo2)
            for hp in range(HP):
                for half in range(2):
                    fcols = slice(half * 512, (half + 1) * 512)
                    zw = psum2.tile([128, 512], f32, tag="bc")
                    nc.tensor.matmul(zw, lhsT=MsbS[:, hp, :],
                                     rhs=wo2S[:, hp, fcols],
                                     start=True, stop=True)
                    nc.scalar.activation(out=wtS[:, hp, fcols], in_=zw,
                                         func=AF.Copy)
        spo_cm.__exit__(None, None, None)

        # late residents (fp8 ffn weights) + main-loop activations
        lr = ctx.enter_context(tc.tile_pool(name="lateres", bufs=1))
        w1S = lr.tile([128, KP1, 2, MLP], f8)
        nc.sync.dma_start(out=w1S, in_=d_w1)
        w2S = lr.tile([128, KP2, 2, DIM], f8)
        nc.sync.dma_start(out=w2S, in_=d_w2)

        # ============ main loop over query blocks ============
        # FFN of block b-1 runs inside iteration b: it gives the tensor
        # engine ~55us of independent DR matmuls to execute while DVE
        # works through block b's LN2 chain (evac/square/derive/recip).
        mpM = ctx.enter_context(tc.tile_pool(name="mmain", bufs=1))

        def emit_ffn1(xh2, g):
            for m in range(MC1):
                z = psum2.tile([128, R], f32, tag="mm")
                for kp in range(KP1):
                    nc.tensor.matmul(
                        z, lhsT=w1S[:, kp, :, m * 128:(m + 1) * 128],
                        rhs=xh2[:, 2 * kp:2 * kp + 2, :],
                        start=(kp == 0), stop=(kp == KP1 - 1), perf_mode=DR)
                nc.scalar.activation(out=g[:, m, :], in_=z, func=AF.Gelu,
                                     bias=b1fS[:, m:m + 1], scale=1.0 / S_W1)

        def emit_ffn2(g, a, cols):
            for n in range(KC):
                z = psum2.tile([128, R], f32, tag="mm")
                for kp in range(KP2):
                    nc.tensor.matmul(
                        z, lhsT=w2S[:, kp, :, n * 128:(n + 1) * 128],
                        rhs=g[:, 2 * kp:2 * kp + 2, :],
                        start=(kp == 0), stop=(kp == KP2 - 1), perf_mode=DR)
                y1 = mpS.tile([128, R], bf16, tag="y1")
                nc.scalar.activation(out=y1, in_=z, func=AF.Identity,
                                     bias=b2S[:, n:n + 1], scale=1.0 / S_W2)
                yo = mpS.tile([128, R], f32, tag="yo")
                nc.gpsimd.tensor_add(out=yo, in0=y1, in1=a[:, n, :])
                nc.sync.dma_start(out=d_out[:, n, cols], in_=yo)

        x, qh = blk0_x, blk0_qh
        prev = None            # (xh2, a, cols) of the previous block
        for blk in range(NBLK):
            cols = slice(blk * R, (blk + 1) * R)

            # attention + wo fold: a = qh @ W_tilde + bo + x
            a = mpM.tile([128, KC, R], bf16, tag=f"a{blk % 2}")
            for m in range(KC):
                z = psum2.tile([128, R], f32, tag="mm")
                for k in range(KC):
                    nc.tensor.matmul(z, lhsT=wtS[:, k, m * 128:(m + 1) * 128],
                                     rhs=qh[:, k, :],
                                     start=(k == 0), stop=(k == KC - 1))
                nc.vector.scalar_tensor_tensor(
                    out=a[:, m, :], in0=z, scalar=boS[:, m:m + 1],
                    in1=x[:, m, :], op0=ADD, op1=ADD)

            g = mpM.tile([128, MC1, R], f8, tag="g")
            if prev is not None:
                emit_ffn1(prev[0], g)

            s1b, s2b = row_stats(lambda c: a[:, c, :], R, mpS, "x2")
            mu2 = mpC.tile([1, R], bf16, tag="mu2")
            nc.vector.tensor_scalar_mul(out=mu2, in0=s1b, scalar1=1.0 / DIM)
            ex2b = mpC.tile([1, R], f32, tag="ex2")
            nc.vector.tensor_scalar_mul(out=ex2b, in0=s2b, scalar1=1.0 / DIM)
            mqb = mpC.tile([1, R], f32, tag="mq")
            nc.vector.tensor_mul(out=mqb, in0=mu2, in1=mu2)
            varb = mpC.tile([1, R], f32, tag="var")
            nc.vector.tensor_sub(out=varb, in0=ex2b, in1=mqb)
            sq2 = mpC.tile([1, R], f32, tag="bsq")
            nc.scalar.activation(out=sq2, in_=varb, func=AF.Sqrt, bias=epsS)
            rstd2 = mpC.tile([1, R], bf16, tag="rs2")
            nc.vector.reciprocal(out=rstd2, in_=sq2)
            mr2 = mpC.tile([1, R], bf16, tag="mr2")
            nc.vector.tensor_mul(out=mr2, in0=mu2, in1=rstd2)

            if prev is not None:
                emit_ffn2(g, prev[1], prev[2])

            r2b = psum1.tile([128, R], f32, tag="r2b")
            nc.tensor.matmul(r2b, lhsT=onerS, rhs=rstd2,
                             start=True, stop=True)
            m2b = psum1.tile([128, R], f32, tag="m2b")
            nc.tensor.matmul(m2b, lhsT=onerS, rhs=mr2,
                             start=True, stop=True)
            # bounce broadcasts to SBUF (negate m2b) so the LN2 apply can
            # run on the otherwise-idle gpsimd engine (no PSUM port there)
            r2bS = mpC.tile([128, R], bf16, tag="r2bS")
            nc.scalar.activation(out=r2bS, in_=r2b, func=AF.Copy)
            m2nS = mpC.tile([128, R], bf16, tag="m2nS")
            nc.scalar.activation(out=m2nS, in_=m2b, func=AF.Copy, scale=-1.0)

            if blk + 1 < NBLK:
                nx, nqh = emit_stats_qproj(blk + 1)

            # xh2 for this block (consumed by FFN1 next iteration)
            xh2 = mpM.tile([128, KC, R], f8, tag="xh2")
            for c in range(KC):
                t = mpC.tile([128, R], bf16, tag="lnt")
                nc.vector.tensor_mul(out=t, in0=a[:, c, :], in1=r2bS)
                nc.vector.tensor_add(out=xh2[:, c, :], in0=t, in1=m2nS)

            prev = (xh2, a, cols)
            if blk + 1 < NBLK:
                x, qh = nx, nqh

        # epilogue: FFN of the last block
        g = mpM.tile([128, MC1, R], f8, tag="g")
        emit_ffn1(prev[0], g)
        emit_ffn2(g, prev[1], prev[2])

    nc.compile()
    return nc


def _prep_inputs(inputs):
    import ml_dtypes

    bf16 = ml_dtypes.bfloat16
    f8 = ml_dtypes.float8_e4m3
    f32 = np.float32

    queries = np.asarray(inputs["queries"], dtype=f32)
    prototypes = np.asarray(inputs["prototypes"], dtype=f32)
    ln1_w = np.asarray(inputs["ln1_w"], dtype=f32)
    ln1_b = np.asarray(inputs["ln1_b"], dtype=f32)
    wq = np.asarray(inputs["wq"], dtype=f32)
    wk = np.asarray(inputs["wk"], dtype=f32)
    wv = np.asarray(inputs["wv"], dtype=f32)
    wo = np.asarray(inputs["wo"], dtype=f32)
    bo = np.asarray(inputs["bo"], dtype=f32)
    ln2_w = np.asarray(inputs["ln2_w"], dtype=f32)
    ln2_b = np.asarray(inputs["ln2_b"], dtype=f32)
    w1 = np.asarray(inputs["w1"], dtype=f32)
    b1 = np.asarray(inputs["b1"], dtype=f32)
    w2 = np.asarray(inputs["w2"], dtype=f32)
    b2 = np.asarray(inputs["b2"], dtype=f32)

    def fm(w):  # [DIM, M] feature-major chunks -> [128, KC, M]
        return np.ascontiguousarray(
            w.reshape(KC, 128, w.shape[1]).transpose(1, 0, 2))

    def cols128(v, nchunks):
        return np.ascontiguousarray(v.reshape(nchunks, 128).T).astype(f32)

    wq_d = fm((wq * ln1_w[:, None]).astype(bf16))
    wk_d = fm((wk * ln1_w[:, None]).astype(bf16))
    wv_d = fm((wv * ln1_w[:, None]).astype(bf16))
    w1_s = np.clip(w1 * ln2_w[:, None] * S_W1, -240, 240).astype(f8)
    w2_s = np.clip(w2 * S_W2, -240, 240).astype(f8)
    w1_d = np.ascontiguousarray(
        w1_s.reshape(KP1, 2, 128, MLP).transpose(2, 0, 1, 3))
    w2_d = np.ascontiguousarray(
        w2_s.reshape(KP2, 2, 128, DIM).transpose(2, 0, 1, 3))
    wo2_d = np.ascontiguousarray(
        wo.astype(bf16).reshape(HP, 128, DIM).transpose(1, 0, 2))

    qc2 = np.stack([-(ln1_w @ wq), ln1_b @ wq]).astype(bf16)
    kc2 = np.stack([-(ln1_w @ wk), ln1_b @ wk]).astype(bf16)
    vc2 = np.stack([-(ln1_w @ wv), ln1_b @ wv]).astype(bf16)
    b1f = (b1 + ln2_b @ w1).astype(f32)

    oha = np.zeros((DIM, HEADS), dtype=f32)
    for h in range(HEADS):
        oha[h * DH:(h + 1) * DH, h] = 1.0
    oha_d = fm(oha.astype(bf16))
    ohb_d = np.ascontiguousarray(oha.T).astype(bf16)

    pT = fm(prototypes.T.astype(bf16))

    common = {
        "pT": pT, "wq_d": wq_d, "wk_d": wk_d, "wv_d": wv_d,
        "wo2_d": wo2_d, "w1_d": w1_d, "w2_d": w2_d,
        "qc2": qc2, "kc2": kc2, "vc2": vc2,
        "b1f_c": cols128(b1f, MC1), "b2_c": cols128(b2, KC),
        "bo_c": cols128(bo, KC), "oha": oha_d, "ohb": ohb_d,
    }
    qT = queries.T.astype(bf16)                    # [DIM, NQ]
    in_maps = []
    for c in range(N_CORES):
        m = dict(common)
        qc = fm(np.ascontiguousarray(qT[:, c * RPC:(c + 1) * RPC]))
        # [128, KC, RPC] -> per-block contiguous [128, NBLK, KC, R]
        m["qT"] = np.ascontiguousarray(
            qc.reshape(128, KC, NBLK, R).transpose(0, 2, 1, 3))
        in_maps.append(m)
    return in_maps


def kernel(**inputs):
    from concourse.bass_utils import run_bass_kernel_spmd

    in_maps = _prep_inputs(inputs)

    if "nc" not in _BUILD_CACHE:
        _BUILD_CACHE["nc"] = _build_nc()
    nc = _BUILD_CACHE["nc"]

    trace = bool(os.environ.get("KERNEL_TRACE"))
    res = run_bass_kernel_spmd(nc, in_maps, core_ids=list(range(N_CORES)),
                               trace=trace)
    _BUILD_CACHE["last_res"] = res
    # yT [128, KC, RPC] -> rows
    parts = []
    for c in range(N_CORES):
        yT = res.results[c]["yT"]
        parts.append(yT.transpose(2, 1, 0).reshape(RPC, DIM))
    return np.ascontiguousarray(np.concatenate(parts, axis=0))
